# revision 48
# baseline (speedup 1.0000x reference)
"""Causal self-attention (B=2, T=2048, C=1024, NH=16, HD=64) on 8 TRN2 cores.

Sharding: core c -> batch b = c//4, head group j = c%4 (4 heads: 4j..4j+3).
Each core computes its batch's QKV projection for its 4 heads, rope, causal
flash-style attention in S^T layout (k on partitions, q on free dim), and a
partial output projection y_part^T = Wp_slice^T.T @ attT in bf16. Host sums
the 4 per-batch partials and adds b_proj.

Schedule: the 8 (head-pair, q-chunk) attention units run hp-interleaved
per qc. Per kt-tile-pair the PE stream is S(pair, priority-0 so the Tile
scheduler keeps the K=64 h0/h64 row-group pairs glued and runs S as early
as deps allow) -> fillers -> PV(prev) so the in-order PE queue never
blocks on the exp; QK projection (B), V projection (C), out-projection
(E) and the rowsum-broadcast matmuls are sliced into filler chunks paced
by an ACT-slack budget with per-tile deadlines. Diagonal kt tiles are
column-trimmed everywhere (S / exp / mask / PV) using the shift identity
mask_d[k, q] == mask_0[k, q-128d]. Rowsum normalization broadcasts
1/rowsum across 64 partitions with a K=1 ones-matmul (no DRAM bounce).
y streams out per qc as bf16 partials. Startup: PE-warmup matmuls hold
the HAM clock at 2.4GHz through the input-DMA wait; x/wqk arrive as
contiguous-per-partition pieces spread over the sync/scalar/gpsimd
queues in need order (gpsimd SWDGE only carries late bulk).
"""
import numpy as np
import ml_dtypes
from contextlib import ExitStack

import concourse.bass as bass
import concourse.mybir as mybir
import concourse.tile as tile
from concourse import bacc
from concourse.bass_utils import run_bass_kernel_spmd

F32 = mybir.dt.float32
BF16 = mybir.dt.bfloat16
FP8 = mybir.dt.float8e4
AF = mybir.ActivationFunctionType
ALU = mybir.AluOpType

B, T, C = 2, 2048, 1024
NH, HD = 16, 64
TL = 2048          # per-core token count (one batch)
NCT = C // 128     # 8 contraction tiles
NTC = TL // 512    # 4 t-chunks of 512
NTT = TL // 128    # 16 token tiles of 128

TRACE = False      # set by test harness for profiling runs
_CACHE = {}

# pacing cost constants (ns) for the filler budget
ACT_FULL = 1166
ACT_DIAG = [1163, 953, 740, 527]
S_COST = [245, 190, 137, 85]
PV_COST = [432, 380, 274, 166]


def _build_nc():
    nc = bacc.Bacc("TRN2", target_bir_lowering=False, debug=False)
    # xT is t-chunk-major and wqk is m-column-major so startup DMA pieces
    # are contiguous per partition (128 segments -> ~0.6us issue; strided
    # slices of a ct-major layout cost 4-6us to issue on the DGE ring)
    xT_d = nc.dram_tensor("xT", [128, NTC, NCT, 512], BF16, kind="ExternalInput").ap()
    wqk_d = nc.dram_tensor("wqkT", [128, 4, NCT, 128], BF16, kind="ExternalInput").ap()
    wv_d = nc.dram_tensor("wvT", [128, NCT, 256], BF16, kind="ExternalInput").ap()
    bqk_d = nc.dram_tensor("bqk", [128, 4], F32, kind="ExternalInput").ap()
    bv_d = nc.dram_tensor("bv", [128, 256], F32, kind="ExternalInput").ap()
    rope_d = nc.dram_tensor("rope", [128, TL], BF16, kind="ExternalInput").ap()
    mask_d = nc.dram_tensor("mask3", [128, 2, 512], BF16, kind="ExternalInput").ap()
    wp_d = nc.dram_tensor("wpT", [128, 2, 1024], BF16, kind="ExternalInput").ap()
    yT_d = nc.dram_tensor("yT", [1024, TL], BF16, kind="ExternalOutput").ap()

    with tile.TileContext(nc) as tc, ExitStack() as ctx:
        sb = ctx.enter_context(tc.tile_pool(name="sb", bufs=1))
        ptp = ctx.enter_context(tc.tile_pool(name="ptp", bufs=8))
        ytp = ctx.enter_context(tc.tile_pool(name="ytp", bufs=8))

        xT = sb.tile([128, NTC, NCT, 512], BF16)
        wqk = sb.tile([128, 4, NCT, 128], BF16)
        wv = sb.tile([128, NCT, 256], BF16)
        bqk = sb.tile([128, 4], F32)
        bv = sb.tile([128, 256], F32)
        rope = sb.tile([128, TL], BF16)
        mask3 = sb.tile([128, 2, 512], BF16)
        wp = sb.tile([128, 2, 1024], BF16)
        q_sb = sb.tile([128, 2, TL], BF16)
        k_sb = sb.tile([128, 2, TL], BF16)
        v_sb = sb.tile([128, 4 * NTT, 65], BF16)
        attT = sb.tile([128, 2, TL], F32)
        attbf = sb.tile([128, 2, TL], BF16)
        rs_sb = sb.tile([128, TL], F32)    # rowsum r_h=64*hp+32*h rows
        rsr_sb = sb.tile([128, TL], F32)   # 1/rowsum (full partitions)
        rsbf = sb.tile([128, TL], BF16)    # bf16 1/rowsum for bcast matmul
        ones_sb = sb.tile([128, 64], BF16)
        warm_sb = sb.tile([128, 64], BF16)
        warm_out = sb.tile([1, 16], F32)

        # ---- memsets: warm tiles on vector (free pre-DMA) so the dummy
        # exp below pulls the ACT_TABLE_LOAD (~2.7us) before the scalar
        # queue starts issuing DMAs; the rest on gpsimd ----
        nc.vector.memset(warm_sb, 0.125)
        nc.vector.memset(ones_sb, 1.0)
        nc.gpsimd.memset(v_sb[:, :, 64:65], 1.0)
        nc.gpsimd.memset(rs_sb, 1.0)  # keep unwritten partitions finite
        nc.scalar.activation(warm_out, warm_sb[0:1, 0:16], AF.Exp,
                             bias=0.0, scale=0.125)

        # ---- input DMA. Critical-path pieces go on sync+scalar (HW DGE;
        # gpsimd SWDGE transfers are slow). wqk is split by m-column so
        # B(m=2 then 0) can start on its slice; x chunks move as ct-halves.
        # Nothing is issued on scalar after startup (it would interleave
        # ~0.7us issues into the exp stream); late bulk rides gpsimd. ----
        def xpiece(q, c, cts):
            q.dma_start(out=xT[:, c, cts, :], in_=xT_d[:, c, cts, :])

        def wqkpiece(q, m):
            q.dma_start(out=wqk[:, m], in_=wqk_d[:, m])

        # every x chunk split across all three queues (the early kernel is
        # input-bandwidth bound; per-queue ~140GB/s); rope chunk0 early
        # every x chunk split across all three queues (the early kernel is
        # input-bandwidth bound; per-queue ~140GB/s); rope chunk0 early
        nc.scalar.dma_start(out=bqk, in_=bqk_d)
        wqkpiece(nc.sync, 2)
        nc.scalar.dma_start(out=rope[:, 0:512], in_=rope_d[:, 0:512])
        xpiece(nc.gpsimd, 0, slice(6, 8))
        xpiece(nc.sync, 0, slice(0, 3))
        wqkpiece(nc.scalar, 0)
        xpiece(nc.scalar, 0, slice(3, 6))
        wqkpiece(nc.gpsimd, 1)
        wqkpiece(nc.sync, 3)
        nc.scalar.dma_start(out=wv, in_=wv_d)
        nc.scalar.dma_start(out=bv, in_=bv_d)
        xpiece(nc.sync, 1, slice(0, 3))
        xpiece(nc.gpsimd, 1, slice(6, 8))
        xpiece(nc.scalar, 1, slice(3, 6))
        nc.sync.dma_start(out=mask3, in_=mask_d)
        nc.sync.dma_start(out=wp, in_=wp_d)
        nc.scalar.dma_start(out=rope[:, 512:TL], in_=rope_d[:, 512:TL])
        for c in (2, 3):
            xpiece(nc.sync, c, slice(0, 3))
            xpiece(nc.scalar, c, slice(3, 6))
            xpiece(nc.gpsimd, c, slice(6, 8))

        with tc.tile_pool(name="pds", bufs=2, space="PSUM") as pds, \
             tc.tile_pool(name="pdv", bufs=1, space="PSUM") as pdv, \
             tc.tile_pool(name="put", bufs=2, space="PSUM") as put:

            # ---- PE warmup: keep HAM busy during the input DMA wait,
            # sized to end roughly when x chunk0 lands so B bursts warm ----
            for g in range(15):
                wps = put.tile([128, 512], F32, tag="u", name=f"warm_{g}")
                for j in range(8):
                    nc.tensor.matmul(
                        wps[0:64, 0:64], warm_sb, warm_sb,
                        start=(j == 0), stop=(j == 7), skip_group_check=True)

            # ---- emission helpers ----
            def emit_b_m(m, c):
                """QK projection m-tile for t-chunk c: 8 MMs + bias*rope."""
                csl = slice(c * 512, (c + 1) * 512)
                ps = put.tile([128, 512], F32, tag="u", name=f"pb_{m}_{c}")
                for ct in range(NCT):
                    nc.tensor.matmul(
                        ps, wqk[:, m, ct, :], xT[:, c, ct, :],
                        start=(ct == 0), stop=(ct == NCT - 1),
                        skip_group_check=True)
                dest = q_sb if m < 2 else k_sb
                nc.vector.scalar_tensor_tensor(
                    out=dest[:, m % 2, csl], in0=ps, scalar=bqk[:, m:m + 1],
                    in1=rope[:, csl], op0=ALU.add, op1=ALU.mult)

            def emit_c_tt(tt):
                """V projection for one 128-token tile (8 MMs + bias add)."""
                ps = put.tile([128, 512], F32, tag="u", name=f"pc_{tt}")[:, 0:256]
                c, r = tt // 4, tt % 4
                for ct in range(NCT):
                    nc.tensor.matmul(
                        ps, xT[:, c, ct, r * 128:(r + 1) * 128], wv[:, ct, :],
                        start=(ct == 0), stop=(ct == NCT - 1),
                        skip_group_check=True)
                nc.vector.tensor_add(
                    v_sb[:, 4 * tt:4 * tt + 4, 0:64],
                    ps.rearrange("p (a b) -> p a b", a=4),
                    bv.rearrange("p (a b) -> p a b", a=4))

            def emit_e_mt(qc, mt):
                """Out-projection tile: 2 accum MMs + copy + y DMA."""
                qsl = slice(qc * 512, (qc + 1) * 512)
                ps = put.tile([128, 512], F32, tag="u", name=f"pe_{qc}_{mt}")
                for hp in range(2):
                    nc.tensor.matmul(
                        ps, wp[:, hp, mt * 128:(mt + 1) * 128], attbf[:, hp, qsl],
                        start=(hp == 0), stop=(hp == 1), skip_group_check=True)
                yt = ytp.tile([128, 512], BF16, tag="yt", name=f"yt_{qc}_{mt}")
                if qc == 3 and mt % 2 == 1:
                    nc.scalar.copy(yt, ps)
                else:
                    nc.vector.tensor_copy(yt, ps)
                # final-qc output alternates sync/scalar so the 8 tail
                # issues (~0.65us each) don't serialize on one queue
                dq = nc.scalar if (qc == 3 and mt % 2 == 1) else nc.sync
                dq.dma_start(out=yT_d[mt * 128:(mt + 1) * 128, qsl], in_=yt)

            units = {}  # ui -> dict(pv=[...])

            def emit_S(t):
                hp, qc, kt, ui = t["hp"], t["qc"], t["kt"], t["ui"]
                o, W = t["o"], t["w"]
                sps = pds.tile([128, 1024], F32, tag="sps",
                               name=f"sps_{ui}_{kt}")
                t["sps"] = sps
                # priority-0: schedule S as early as deps allow so the exp
                # stream never starves and the h0/h1 row-group pair stays
                # glued (the scheduler's gap-filling otherwise splits it,
                # losing the concurrent-pair timing)
                with tc.high_priority():
                    for h in range(2):
                        hsl = slice(h * 64, (h + 1) * 64)
                        nc.tensor.matmul(
                            sps[:, h * 512 + o:(h + 1) * 512],
                            k_sb[hsl, hp, kt * 128:(kt + 1) * 128],
                            q_sb[hsl, hp, qc * 512 + o:(qc + 1) * 512],
                            start=True, stop=True, skip_group_check=True)

            def emit_exp(t):
                o = t["o"]
                pt = ptp.tile([128, 1024], BF16, tag="pt",
                              name=f"pt_{t['ui']}_{t['kt']}")
                t["pt"] = pt
                src = t["sps"].rearrange("p (a b) -> p a b", a=2)[:, :, o:512]
                dst = pt.rearrange("p (a b) -> p a b", a=2)[:, :, o:512]
                nc.scalar.activation(dst, src, AF.Exp, bias=0.0, scale=0.125)
                if t["d"] is not None:
                    nc.vector.tensor_mul(dst, dst, mask3[:, :, 0:t["w"]])

            def emit_PV(t):
                hp, qc, kt, ui = t["hp"], t["qc"], t["kt"], t["ui"]
                o, nkt = t["o"], 4 * (qc + 1)
                if t["first"]:
                    units[ui] = [pdv.tile([65, 512], F32, tag=f"pv{h}",
                                          name=f"pv_{ui}_{h}")
                                 for h in range(2)]
                pv = units[ui]
                for h in range(2):
                    u = 4 * kt + 2 * hp + h
                    nc.tensor.matmul(
                        pv[h][:, o:512], v_sb[:, u, :],
                        t["pt"][:, h * 512 + o:(h + 1) * 512],
                        start=(kt == 0), stop=(kt == nkt - 1),
                        skip_group_check=True)

            def emit_norm1(ui, hp, qc):
                """pv evacuation + reciprocal (all DVE)."""
                qsl = slice(qc * 512, (qc + 1) * 512)
                pv = units[ui]
                for h in range(2):
                    r = 64 * hp + 32 * h
                    nc.vector.tensor_copy(
                        attT[h * 64:(h + 1) * 64, hp, qsl], pv[h][0:64, :])
                    nc.vector.tensor_copy(
                        rs_sb[r:r + 1, qsl], pv[h][64:65, :])
                nc.vector.reciprocal_approx_fast(
                    rsr_sb[:, qsl], rs_sb[:, qsl])
                nc.vector.tensor_copy(
                    rsbf[64 * hp:64 * hp + 33, qsl],
                    rsr_sb[64 * hp:64 * hp + 33, qsl])

            def emit_norm2(hp, qc):
                """K=1 ones-matmul broadcast of 1/rowsum + scale to attbf."""
                qsl = slice(qc * 512, (qc + 1) * 512)
                nps = put.tile([128, 512], F32, tag="u", name=f"nps_{hp}_{qc}")
                for h in range(2):
                    r = 64 * hp + 32 * h
                    nc.tensor.matmul(
                        nps[h * 64:(h + 1) * 64, :],
                        ones_sb[r:r + 1, 0:64], rsbf[r:r + 1, qsl],
                        start=True, stop=True, skip_group_check=True,
                        tile_position=(r, 64 * h))
                nc.vector.tensor_mul(
                    attbf[:, hp, qsl], attT[:, hp, qsl], nps)

            # ---- global tile list over hp-interleaved qc units ----
            tiles = []
            UNITS = [(hp, qc) for qc in range(4) for hp in (0, 1)]
            unit_start = {}
            for ui, (hp, qc) in enumerate(UNITS):
                unit_start[ui] = len(tiles)
                nkt = 4 * (qc + 1)
                for kt in range(nkt):
                    d = kt - 4 * qc if kt >= 4 * qc else None
                    tiles.append(dict(
                        ui=ui, hp=hp, qc=qc, kt=kt, d=d,
                        o=0 if d is None else 128 * d,
                        w=512 if d is None else 512 - 128 * d,
                        first=(kt == 0), last=(kt == nkt - 1)))

            def gidx(hp, qc, kt):
                ui = UNITS.index((hp, qc))
                return unit_start[ui] + kt

            # ---- filler list: (cost, deadline, not_before, emit) ----
            fillers = []

            def add_filler(cost, deadline, not_before, fn):
                fillers.append([cost, deadline, not_before, fn])

            NEVER = 10 ** 9
            # B chunks: (hp, c) ready after chunk c lands; due before unit
            # (hp, qc=c) starts. k m-tile first (S needs k cols of chunk c
            # only for qc>=c... both k and q of chunk c needed at qc=c).
            for c in range(1, 4):
                for hp in (0, 1):
                    dl = gidx(hp, c, 0)
                    nb = gidx(0, max(c - 1, 0), 0)
                    for m in (2 + hp, 0 + hp):
                        add_filler(1730, dl, nb, lambda m=m, c=c: emit_b_m(m, c))
            # C tiles tt: due before PV of (hp0, qc=tt//4, kt=tt) which is
            # emitted at iteration gidx+1.
            for tt in range(2, NTT):
                qc = tt // 4
                dl = gidx(0, qc, tt) + 1
                nb = gidx(0, max(qc - 1, 0), 0)
                add_filler(880, dl, nb, lambda tt=tt: emit_c_tt(tt))
            # norm2 + E chunks are appended dynamically after each unit ends.

            # upfront work (paced by first DMA chunk); C tt0/tt1 as fillers
            # with tight deadlines so the first exp fires right after B(hp0)
            emit_b_m(2, 0)
            emit_b_m(0, 0)
            add_filler(880, 1, 0, lambda: emit_c_tt(0))
            add_filler(880, 2, 0, lambda: emit_c_tt(1))
            add_filler(1730, gidx(1, 0, 0), 0, lambda: emit_b_m(3, 0))
            add_filler(1730, gidx(1, 0, 0), 0, lambda: emit_b_m(1, 0))

            def pop_forced(i, pv_stage):
                for f in list(fillers):
                    if f[1] <= i:
                        fillers.remove(f)
                        f[3]()

            def pop_budget(i, surplus):
                while surplus[0] > 0:
                    pick = None
                    for f in fillers:
                        if f[2] <= i and f[0] <= surplus[0]:
                            pick = f
                            break
                    if pick is None:
                        break
                    fillers.remove(pick)
                    surplus[0] -= pick[0]
                    pick[3]()

            # ---- main pipelined loop ----
            surplus = [1500]
            prev = None
            for i, t in enumerate(tiles):
                pop_forced(i, False)
                if t["kt"] % 2 == 0:
                    # kt-pair S batching: both tiles' S pairs back-to-back
                    # (fewer weight-path transitions on the PE stream)
                    emit_S(t)
                    emit_S(tiles[i + 1])
                emit_exp(t)
                d = t["d"]
                surplus[0] += (ACT_FULL if d is None else ACT_DIAG[d]) \
                    - (S_COST[0] if d is None else S_COST[d])
                pop_budget(i, surplus)
                if prev is not None:
                    emit_PV(prev)
                    pd = prev["d"]
                    surplus[0] -= PV_COST[0] if pd is None else PV_COST[pd]
                    if prev["last"]:
                        ui, hp, qc = prev["ui"], prev["hp"], prev["qc"]
                        emit_norm1(ui, hp, qc)
                        add_filler(470, NEVER, i + 2,
                                   lambda hp=hp, qc=qc: emit_norm2(hp, qc))
                        if hp == 1:  # both hps of qc done -> out-projection
                            for mt in range(NTT // 2):
                                add_filler(500, NEVER, i + 3,
                                           lambda qc=qc, mt=mt: emit_e_mt(qc, mt))
                prev = t

            # ---- tail ----
            for f in list(fillers):   # flush whatever filler work remains
                fillers.remove(f)
                f[3]()
            emit_PV(prev)
            emit_norm1(prev["ui"], prev["hp"], prev["qc"])
            emit_norm2(1, 3)
            for mt in range(NTT // 2):
                emit_e_mt(3, mt)
    nc.compile()
    return nc


def _rope_T():
    theta = 1.0 / (10000.0 ** (2.0 * np.arange(0, HD // 2, dtype=np.float32) / HD))
    seq = np.arange(1, T + 1, dtype=np.float32)
    ang = np.einsum('n,d->nd', seq, theta)
    ang = np.concatenate([ang, ang], axis=-1)
    f = (np.cos(ang) + np.sin(ang)).astype(np.float32)  # [T, 64]
    return np.concatenate([f.T, f.T], axis=0)           # [128, T]


def _host_inputs(x, W_attn, b_attn, W_proj, b_proj):
    bf = ml_dtypes.bfloat16
    ropeT = _rope_T().astype(bf)
    kp = np.arange(128)[:, None]
    qf = np.arange(512)[None, :]
    m0 = (kp <= qf).astype(np.float32).astype(bf)       # [128, 512]
    mask3 = np.ascontiguousarray(
        np.broadcast_to(m0[:, None, :], (128, 2, 512)))

    in_maps = []
    for c in range(8):
        b, j = divmod(c, 4)
        hs = [4 * j + i for i in range(4)]
        xT = np.ascontiguousarray(x[b].T).astype(bf)          # [1024, TL]
        q_rows = np.concatenate([W_attn[64 * h:64 * (h + 1)] for h in hs], 0)
        k_rows = np.concatenate([W_attn[C + 64 * h:C + 64 * (h + 1)] for h in hs], 0)
        WqkT = np.concatenate([q_rows, k_rows], 0).T          # [1024, 512]
        bqk = np.concatenate(
            [np.concatenate([b_attn[64 * h:64 * (h + 1)] for h in hs]),
             np.concatenate([b_attn[C + 64 * h:C + 64 * (h + 1)] for h in hs])])
        v_rows = np.concatenate([W_attn[2 * C + 64 * h:2 * C + 64 * (h + 1)] for h in hs], 0)
        WvT = v_rows.T                                        # [1024, 256]
        bv = np.concatenate([b_attn[2 * C + 64 * h:2 * C + 64 * (h + 1)] for h in hs])
        WpT = np.concatenate([W_proj[:, 64 * h:64 * (h + 1)] for h in hs], 1).T  # [256,1024]
        in_maps.append({
            "xT": np.ascontiguousarray(
                xT.reshape(NCT, 128, NTC, 512).transpose(1, 2, 0, 3)),
            "wqkT": np.ascontiguousarray(
                WqkT.astype(bf).reshape(NCT, 128, 4, 128).transpose(1, 2, 0, 3)),
            "wvT": np.ascontiguousarray(
                WvT.astype(bf).reshape(NCT, 128, 256).transpose(1, 0, 2)),
            "bqk": np.ascontiguousarray(bqk.reshape(4, 128).T.astype(np.float32)),
            "bv": np.ascontiguousarray(
                np.broadcast_to(bv[None, :].astype(np.float32), (128, 256))),
            "rope": ropeT,
            "mask3": mask3,
            "wpT": np.ascontiguousarray(
                WpT.astype(bf).reshape(2, 128, 1024).transpose(1, 0, 2)),
        })
    return in_maps


def kernel(x, W_attn, b_attn, W_proj, b_proj):
    if "nc" not in _CACHE:
        _CACHE["nc"] = _build_nc()
    nc = _CACHE["nc"]
    in_maps = _host_inputs(x, W_attn, b_attn, W_proj, b_proj)
    res = run_bass_kernel_spmd(nc, in_maps, list(range(8)), trace=TRACE)
    _CACHE["last"] = res
    y = np.zeros((B, T, C), np.float32)
    for c in range(8):
        y[c // 4] += res.results[c]["yT"].astype(np.float32).T
    y += b_proj.astype(np.float32)
    return y
